# revision 1
# baseline (speedup 1.0000x reference)
"""MemN2N block kernel for 8 TRN2 NeuronCores.

Reference computation (per batch b):
    m      = story_a @ u^T          # (M, Q)  contraction over D
    p      = softmax(m, axis=Q)     # (M, Q)  softmax over Q (free axis)
    c      = p^T @ story_c          # (Q, D)  contraction over M
    out    = c + u @ H              # (Q, D)

Sharding: data-parallel over batch B=32 -> 4 batches per core, no
collectives.  Per core, per batch, stream M=8192 in 128-row tiles:
  - PE transpose of the story_a tile (D onto partitions)
  - mm1: scores(m,q) = aT.T @ uT
  - ACT: p = exp(scores) fused with row-sum (softmax over q needs no
    max-subtraction: |scores| <~ 70 so exp stays in fp32 range)
  - DVE: reciprocal; ACT: p *= 1/z
  - mm2: acc(d,q) += c_tile.T @ p   (PSUM accumulation over all tiles)
  - mm3: acc(d,q) += H.T @ uT       (the u @ H residual, same PSUM group)
  - final PE transpose of acc -> (q,d), DMA out.

The M rows are processed in a permuted order (m = p*64 + n) so each DMA
chunk is 16KB-contiguous per partition; softmax is per-row and the
weighted sum over M is order-invariant, so the permutation cancels as
long as story_a and story_c use the same one.
"""

import numpy as np

B, M, Q, D = 32, 8192, 64, 128
N_CORES = 8
BPC = B // N_CORES          # batches per core
P = 128                     # partitions / m-tile rows
NT = M // P                 # 64 m-subtiles per batch
CHUNK = 32                  # m-subtiles per DMA chunk
NCH = NT // CHUNK           # chunks per batch

_cache = {}


def _build():
    import concourse.bass as bass
    import concourse.tile as tile
    from concourse import bacc, mybir
    from concourse.masks import make_identity

    f32 = mybir.dt.float32
    nc = bacc.Bacc("TRN2", target_bir_lowering=False, debug=False,
                   num_devices=N_CORES)

    a_ap = nc.dram_tensor("story_a", [BPC, M, D], f32, kind="ExternalInput").ap()
    u_ap = nc.dram_tensor("u", [BPC, Q, D], f32, kind="ExternalInput").ap()
    c_ap = nc.dram_tensor("story_c", [BPC, M, D], f32, kind="ExternalInput").ap()
    h_ap = nc.dram_tensor("H", [D, D], f32, kind="ExternalInput").ap()
    o_ap = nc.dram_tensor("out", [BPC, Q, D], f32, kind="ExternalOutput").ap()

    Exp = mybir.ActivationFunctionType.Exp

    with tile.TileContext(nc) as tc:
        with (
            tc.tile_pool(name="consts", bufs=1) as consts,
            tc.tile_pool(name="io", bufs=2) as io,
            tc.tile_pool(name="work", bufs=3) as work,
            tc.tile_pool(name="small", bufs=4) as small,
            tc.tile_pool(name="ps_t", bufs=2, space="PSUM") as ps_t,
            tc.tile_pool(name="ps_s", bufs=2, space="PSUM") as ps_s,
            tc.tile_pool(name="ps_acc", bufs=2, space="PSUM") as ps_acc,
            tc.tile_pool(name="ps_misc", bufs=1, space="PSUM") as ps_misc,
        ):
            ident = consts.tile([P, P], f32)
            make_identity(nc, ident)
            h_sb = consts.tile([D, D], f32)
            nc.sync.dma_start(h_sb, h_ap)

            for b in range(BPC):
                # u tile and its transpose uT (D on partitions)
                u_sb = small.tile([Q, D], f32, tag="u_sb")
                nc.sync.dma_start(u_sb, u_ap[b])
                uT_ps = ps_misc.tile([D, Q], f32, tag="uT_ps")
                nc.tensor.transpose(uT_ps, u_sb, ident[:Q, :Q])
                uT_sb = small.tile([D, Q], f32, tag="uT_sb")
                nc.vector.tensor_copy(uT_sb, uT_ps)

                # (M, D) viewed as (p, n, d) with m = p*NT + n
                a_r = a_ap[b].rearrange("(p n) d -> p n d", p=P)
                c_r = c_ap[b].rearrange("(p n) d -> p n d", p=P)

                acc_ps = ps_acc.tile([D, Q], f32, tag="acc")

                for ch in range(NCH):
                    ach = io.tile([P, CHUNK, D], f32, tag="ach")
                    nc.sync.dma_start(ach, a_r[:, ch * CHUNK:(ch + 1) * CHUNK, :])
                    cch = io.tile([P, CHUNK, D], f32, tag="cch")
                    nc.sync.dma_start(cch, c_r[:, ch * CHUNK:(ch + 1) * CHUNK, :])

                    for j in range(CHUNK):
                        idx = ch * CHUNK + j
                        at_ps = ps_t.tile([D, P], f32, tag="at_ps")
                        nc.tensor.transpose(at_ps, ach[:, j, :], ident)
                        at_sb = work.tile([D, P], f32, tag="at_sb")
                        nc.vector.tensor_copy(at_sb, at_ps)

                        s_ps = ps_s.tile([P, Q], f32, tag="s_ps")
                        nc.tensor.matmul(s_ps, at_sb, uT_sb,
                                         start=True, stop=True)

                        p_sb = work.tile([P, Q], f32, tag="p_sb")
                        z = small.tile([P, 1], f32, tag="z")
                        nc.scalar.activation(p_sb, s_ps, Exp, accum_out=z)
                        zi = small.tile([P, 1], f32, tag="zi")
                        nc.vector.reciprocal(zi, z)
                        nc.scalar.mul(p_sb, p_sb, zi)

                        nc.tensor.matmul(acc_ps, cch[:, j, :], p_sb,
                                         start=(idx == 0), stop=False)

                # residual: acc += H.T @ uT  == (u @ H)^T
                nc.tensor.matmul(acc_ps, h_sb, uT_sb, start=False, stop=True)

                acc_sb = work.tile([D, Q], f32, tag="acc_sb")
                nc.scalar.copy(acc_sb, acc_ps)
                oT_ps = ps_misc.tile([Q, D], f32, tag="oT_ps")
                nc.tensor.transpose(oT_ps, acc_sb, ident)
                o_sb = work.tile([Q, D], f32, tag="o_sb")
                nc.vector.tensor_copy(o_sb, oT_ps)
                nc.sync.dma_start(o_ap[b], o_sb)

    nc.compile()
    return nc


def _get_nc():
    if "nc" not in _cache:
        _cache["nc"] = _build()
    return _cache["nc"]


def _in_maps(story_a, u, story_c, H):
    story_a = np.ascontiguousarray(story_a, dtype=np.float32)
    u = np.ascontiguousarray(u, dtype=np.float32)
    story_c = np.ascontiguousarray(story_c, dtype=np.float32)
    H = np.ascontiguousarray(H, dtype=np.float32)
    maps = []
    for i in range(N_CORES):
        s = slice(i * BPC, (i + 1) * BPC)
        maps.append({
            "story_a": story_a[s],
            "u": u[s],
            "story_c": story_c[s],
            "H": H,
        })
    return maps


def kernel(story_a, u, story_c, H):
    from concourse.bass_utils import run_bass_kernel_spmd

    nc = _get_nc()
    maps = _in_maps(story_a, u, story_c, H)
    res = run_bass_kernel_spmd(nc, maps, core_ids=list(range(N_CORES)))
    out = np.concatenate([res.results[i]["out"] for i in range(N_CORES)],
                         axis=0)
    return out.astype(np.float32)


# revision 2
# speedup vs baseline: 4.6833x; 4.6833x over previous
"""MemN2N block kernel for 8 TRN2 NeuronCores.

Reference computation (per batch b):
    m      = story_a @ u^T          # (M, Q)  contraction over D
    p      = softmax(m, axis=Q)     # (M, Q)  softmax over Q (free axis)
    c      = p^T @ story_c          # (Q, D)  contraction over M
    out    = c + u @ H              # (Q, D)

Sharding: data-parallel over batch B=32 -> 4 batches per core, no
collectives.  Per core, per batch, stream M=8192 in 128-row tiles:
  - PE transpose of the story_a tile (D onto partitions)
  - mm1: scores(m,q) = aT.T @ uT
  - ACT: p = exp(scores) fused with row-sum (softmax over q needs no
    max-subtraction: |scores| <~ 70 so exp stays in fp32 range)
  - DVE: reciprocal; ACT: p *= 1/z
  - mm2: acc(d,q) += c_tile.T @ p   (PSUM accumulation over all tiles)
  - mm3: acc(d,q) += H.T @ uT       (the u @ H residual, same PSUM group)
  - final PE transpose of acc -> (q,d), DMA out.

The M rows are processed in a permuted order (m = p*64 + n) so each DMA
chunk is 16KB-contiguous per partition; softmax is per-row and the
weighted sum over M is order-invariant, so the permutation cancels as
long as story_a and story_c use the same one.
"""

import numpy as np

B, M, Q, D = 32, 8192, 64, 128
N_CORES = 8
BPC = B // N_CORES          # batches per core
P = 128                     # partitions / m-tile rows
NT = M // P                 # 64 m-subtiles per batch
CHUNK = 32                  # m-subtiles per DMA chunk
NCH = NT // CHUNK           # chunks per batch

_cache = {}


def _emit_body(nc, tc, pools, aps):
    """Emit one full pass of the per-core computation."""
    from concourse import mybir

    f32 = mybir.dt.float32
    Exp = mybir.ActivationFunctionType.Exp
    consts, io, work, small, ps_t, ps_s, ps_acc, ps_misc = pools
    a_ap, u_ap, c_ap, h_ap, o_ap, ident, h_sb = aps

    for b in range(BPC):
        # u tile and its transpose uT (D on partitions)
        u_sb = small.tile([Q, D], f32, tag="u_sb")
        nc.sync.dma_start(u_sb, u_ap[b])
        uT_ps = ps_misc.tile([D, Q], f32, tag="uT_ps")
        nc.tensor.transpose(uT_ps, u_sb, ident[:Q, :Q])
        uT_sb = small.tile([D, Q], f32, tag="uT_sb")
        nc.vector.tensor_copy(uT_sb, uT_ps)

        # (M, D) viewed as (p, n, d) with m = p*NT + n
        a_r = a_ap[b].rearrange("(p n) d -> p n d", p=P)
        c_r = c_ap[b].rearrange("(p n) d -> p n d", p=P)

        acc_ps = ps_acc.tile([D, Q], f32, tag="acc")

        for ch in range(NCH):
            ach = io.tile([P, CHUNK, D], f32, tag="ach")
            nc.sync.dma_start(ach, a_r[:, ch * CHUNK:(ch + 1) * CHUNK, :])
            cch = io.tile([P, CHUNK, D], f32, tag="cch")
            nc.sync.dma_start(cch, c_r[:, ch * CHUNK:(ch + 1) * CHUNK, :])

            for j in range(CHUNK):
                idx = ch * CHUNK + j
                at_ps = ps_t.tile([D, P], f32, tag="at_ps")
                nc.tensor.transpose(at_ps, ach[:, j, :], ident)
                at_sb = work.tile([D, P], f32, tag="at_sb")
                nc.vector.tensor_copy(at_sb, at_ps)

                s_ps = ps_s.tile([P, Q], f32, tag="s_ps")
                nc.tensor.matmul(s_ps, at_sb, uT_sb, start=True, stop=True)

                p_sb = work.tile([P, Q], f32, tag="p_sb")
                z = small.tile([P, 1], f32, tag="z")
                nc.scalar.activation(p_sb, s_ps, Exp, accum_out=z)
                zi = small.tile([P, 1], f32, tag="zi")
                nc.vector.reciprocal(zi, z)
                nc.scalar.mul(p_sb, p_sb, zi)

                nc.tensor.matmul(acc_ps, cch[:, j, :], p_sb,
                                 start=(idx == 0), stop=False)

        # residual: acc += H.T @ uT  == (u @ H)^T
        nc.tensor.matmul(acc_ps, h_sb, uT_sb, start=False, stop=True)

        acc_sb = work.tile([D, Q], f32, tag="acc_sb")
        nc.scalar.copy(acc_sb, acc_ps)
        oT_ps = ps_misc.tile([Q, D], f32, tag="oT_ps")
        nc.tensor.transpose(oT_ps, acc_sb, ident)
        o_sb = work.tile([Q, D], f32, tag="o_sb")
        nc.vector.tensor_copy(o_sb, oT_ps)
        nc.sync.dma_start(o_ap[b], o_sb)


def _build(repeat=1):
    import concourse.tile as tile
    from concourse import bacc, mybir
    from concourse.masks import make_identity

    f32 = mybir.dt.float32
    nc = bacc.Bacc("TRN2", target_bir_lowering=False, debug=False,
                   num_devices=N_CORES)

    a_ap = nc.dram_tensor("story_a", [BPC, M, D], f32, kind="ExternalInput").ap()
    u_ap = nc.dram_tensor("u", [BPC, Q, D], f32, kind="ExternalInput").ap()
    c_ap = nc.dram_tensor("story_c", [BPC, M, D], f32, kind="ExternalInput").ap()
    h_ap = nc.dram_tensor("H", [D, D], f32, kind="ExternalInput").ap()
    o_ap = nc.dram_tensor("out", [BPC, Q, D], f32, kind="ExternalOutput").ap()

    with tile.TileContext(nc) as tc:
        with (
            tc.tile_pool(name="consts", bufs=1) as consts,
            tc.tile_pool(name="io", bufs=2) as io,
            tc.tile_pool(name="work", bufs=3) as work,
            tc.tile_pool(name="small", bufs=4) as small,
            tc.tile_pool(name="ps_t", bufs=2, space="PSUM") as ps_t,
            tc.tile_pool(name="ps_s", bufs=2, space="PSUM") as ps_s,
            tc.tile_pool(name="ps_acc", bufs=2, space="PSUM") as ps_acc,
            tc.tile_pool(name="ps_misc", bufs=1, space="PSUM") as ps_misc,
        ):
            ident = consts.tile([P, P], f32)
            make_identity(nc, ident)
            h_sb = consts.tile([D, D], f32)
            nc.sync.dma_start(h_sb, h_ap)

            pools = (consts, io, work, small, ps_t, ps_s, ps_acc, ps_misc)
            aps = (a_ap, u_ap, c_ap, h_ap, o_ap, ident, h_sb)
            for _ in range(repeat):
                _emit_body(nc, tc, pools, aps)

    nc.compile()
    return nc


def _get_nc(repeat=1):
    key = ("nc", repeat)
    if key not in _cache:
        _cache[key] = _build(repeat)
    return _cache[key]


def _in_maps(story_a, u, story_c, H):
    story_a = np.ascontiguousarray(story_a, dtype=np.float32)
    u = np.ascontiguousarray(u, dtype=np.float32)
    story_c = np.ascontiguousarray(story_c, dtype=np.float32)
    H = np.ascontiguousarray(H, dtype=np.float32)
    maps = []
    for i in range(N_CORES):
        s = slice(i * BPC, (i + 1) * BPC)
        maps.append({
            "story_a": story_a[s],
            "u": u[s],
            "story_c": story_c[s],
            "H": H,
        })
    return maps


def kernel(story_a, u, story_c, H):
    from concourse.bass_utils import run_bass_kernel_spmd

    nc = _get_nc()
    maps = _in_maps(story_a, u, story_c, H)
    res = run_bass_kernel_spmd(nc, maps, core_ids=list(range(N_CORES)))
    out = np.concatenate([res.results[i]["out"] for i in range(N_CORES)],
                         axis=0)
    return out.astype(np.float32)


# revision 4
# speedup vs baseline: 6.4496x; 1.3771x over previous
"""MemN2N block kernel for 8 TRN2 NeuronCores.

Reference computation (per batch b):
    m      = story_a @ u^T          # (M, Q)  contraction over D
    p      = softmax(m, axis=Q)     # (M, Q)  softmax over Q (free axis)
    c      = p^T @ story_c          # (Q, D)  contraction over M
    out    = c + u @ H              # (Q, D)

Sharding: data-parallel over batch B=32 -> 4 batches per core, no
collectives.  Per core the 4 batches are processed INTERLEAVED: each
"super-tile" step handles one 128-row m-tile from each batch, so the
ACT/DVE softmax ops cover (128, 4, 64) at once (4x fewer instructions,
amortizing the ~200ns fixed cost per op that otherwise dominates).

Per super-tile:
  - 4x PE transpose of story_a tiles into one PSUM bank (D onto parts)
  - one ACT copy PSUM->SBUF of all 4 transposed tiles
  - 4x mm1: scores[:, b, :] = aT_b.T @ uT_b
  - one ACT exp over (128, 4, 64)  (softmax over q needs no
    max-subtraction: |scores| <~ 70 keeps exp inside fp32 range)
  - one DVE reduce_sum -> Z (128, 4), one DVE reciprocal,
    one DVE broadcast-multiply to normalize p
  - 4x mm2: acc[:, b, :] += c_tile_b.T @ p_b   (PSUM accumulation)
Tail per batch: mm3 acc += H.T @ uT (the u @ H residual, same PSUM
accumulation group), PE transpose of acc -> (q, d), DMA out.

M rows are processed in a permuted order (m = p*64 + n) so each DMA
chunk is 8KB-contiguous per partition; softmax is per-row and the
weighted sum over M is order-invariant, so the permutation cancels as
long as story_a and story_c use the same one.
"""

import numpy as np

B, M, Q, D = 32, 8192, 64, 128
N_CORES = 8
BPC = B // N_CORES          # batches per core
P = 128                     # partitions / m-tile rows
NT = M // P                 # 64 m-subtiles per batch
CHUNK = 16                  # m-subtiles per DMA chunk
NCH = NT // CHUNK           # chunks per batch

_cache = {}


def _emit_body(nc, tc, pools, aps):
    """Emit one full pass of the per-core computation."""
    from concourse import mybir

    f32 = mybir.dt.float32
    Exp = mybir.ActivationFunctionType.Exp
    consts, io, work, small, ps_t, ps_s, ps_acc, ps_misc = pools
    a_ap, u_ap, c_ap, h_ap, o_ap, ident, h_sb = aps

    # u tiles and their transposes uT_b (D on partitions), loaded once
    uT_sbs = []
    for b in range(BPC):
        u_sb = small.tile([Q, D], f32, tag="u_sb")
        nc.sync.dma_start(u_sb, u_ap[b])
        uT_ps = ps_misc.tile([D, Q], f32, tag="uT_ps")
        nc.tensor.transpose(uT_ps, u_sb, ident[:Q, :Q])
        uT_sb = small.tile([D, Q], f32, tag=f"uT_sb{b}")
        nc.vector.tensor_copy(uT_sb, uT_ps)
        uT_sbs.append(uT_sb)

    # (M, D) viewed as (p, n, d) with m = p*NT + n
    a_rs = [a_ap[b].rearrange("(p n) d -> p n d", p=P) for b in range(BPC)]
    c_rs = [c_ap[b].rearrange("(p n) d -> p n d", p=P) for b in range(BPC)]

    acc_ps = ps_acc.tile([P, BPC, Q], f32, tag="acc")

    for ch in range(NCH):
        achs, cchs = [], []
        for b in range(BPC):
            ach = io.tile([P, CHUNK, D], f32, tag=f"ach{b}")
            nc.sync.dma_start(ach, a_rs[b][:, ch * CHUNK:(ch + 1) * CHUNK, :])
            achs.append(ach)
            cch = io.tile([P, CHUNK, D], f32, tag=f"cch{b}")
            nc.sync.dma_start(cch, c_rs[b][:, ch * CHUNK:(ch + 1) * CHUNK, :])
            cchs.append(cch)

        for j in range(CHUNK):
            idx = ch * CHUNK + j
            at_ps = ps_t.tile([P, BPC, P], f32, tag="at_ps")
            for b in range(BPC):
                nc.tensor.transpose(at_ps[:, b, :], achs[b][:, j, :], ident)
            at_sb = work.tile([P, BPC, P], f32, tag="at_sb")
            nc.scalar.copy(at_sb, at_ps)

            s_ps = ps_s.tile([P, BPC, Q], f32, tag="s_ps")
            for b in range(BPC):
                nc.tensor.matmul(s_ps[:, b, :], at_sb[:, b, :], uT_sbs[b],
                                 start=True, stop=True)

            p_sb = work.tile([P, BPC, Q], f32, tag="p_sb")
            nc.scalar.activation(p_sb, s_ps, Exp)
            z = small.tile([P, BPC], f32, tag="z")
            nc.vector.tensor_reduce(z, p_sb, mybir.AxisListType.X,
                                    mybir.AluOpType.add)
            zi = small.tile([P, BPC], f32, tag="zi")
            nc.vector.reciprocal(zi, z)
            nc.vector.tensor_tensor(p_sb, p_sb,
                                    zi[:, :, None].to_broadcast(p_sb.shape),
                                    mybir.AluOpType.mult)

            for b in range(BPC):
                # PSUM accumulation-group state is per bank: only the
                # very first matmul on this bank may use start=True
                # (interleaved per-batch groups reset each other's
                # has_written bits and drop contributions).  The first
                # write of every other batch region lands on cleared
                # has_written bits and overwrites, so start=False is
                # correct there too.
                nc.tensor.matmul(acc_ps[:, b, :], cchs[b][:, j, :],
                                 p_sb[:, b, :],
                                 start=(idx == 0 and b == 0), stop=False,
                                 skip_group_check=True)

    # residual: acc_b += H.T @ uT_b  == (u_b @ H)^T
    for b in range(BPC):
        nc.tensor.matmul(acc_ps[:, b, :], h_sb, uT_sbs[b],
                         start=False, stop=(b == BPC - 1),
                         skip_group_check=True)

    acc_sb = work.tile([P, BPC, Q], f32, tag="acc_sb")
    nc.scalar.copy(acc_sb, acc_ps)
    oT_ps = ps_misc.tile([Q, BPC, D], f32, tag="oT_ps")
    for b in range(BPC):
        nc.tensor.transpose(oT_ps[:, b, :], acc_sb[:, b, :], ident)
    o_sb = work.tile([Q, BPC, D], f32, tag="o_sb")
    nc.vector.tensor_copy(o_sb, oT_ps)
    for b in range(BPC):
        nc.sync.dma_start(o_ap[b], o_sb[:, b, :])


def _build(repeat=1):
    import concourse.tile as tile
    from concourse import bacc, mybir
    from concourse.masks import make_identity

    f32 = mybir.dt.float32
    nc = bacc.Bacc("TRN2", target_bir_lowering=False, debug=False,
                   num_devices=N_CORES)

    a_ap = nc.dram_tensor("story_a", [BPC, M, D], f32, kind="ExternalInput").ap()
    u_ap = nc.dram_tensor("u", [BPC, Q, D], f32, kind="ExternalInput").ap()
    c_ap = nc.dram_tensor("story_c", [BPC, M, D], f32, kind="ExternalInput").ap()
    h_ap = nc.dram_tensor("H", [D, D], f32, kind="ExternalInput").ap()
    o_ap = nc.dram_tensor("out", [BPC, Q, D], f32, kind="ExternalOutput").ap()

    with tile.TileContext(nc) as tc:
        with (
            tc.tile_pool(name="consts", bufs=1) as consts,
            tc.tile_pool(name="io", bufs=2) as io,
            tc.tile_pool(name="work", bufs=3) as work,
            tc.tile_pool(name="small", bufs=4) as small,
            tc.tile_pool(name="ps_t", bufs=2, space="PSUM") as ps_t,
            tc.tile_pool(name="ps_s", bufs=2, space="PSUM") as ps_s,
            tc.tile_pool(name="ps_acc", bufs=2, space="PSUM") as ps_acc,
            tc.tile_pool(name="ps_misc", bufs=1, space="PSUM") as ps_misc,
        ):
            ident = consts.tile([P, P], f32)
            make_identity(nc, ident)
            h_sb = consts.tile([D, D], f32)
            nc.sync.dma_start(h_sb, h_ap)

            pools = (consts, io, work, small, ps_t, ps_s, ps_acc, ps_misc)
            aps = (a_ap, u_ap, c_ap, h_ap, o_ap, ident, h_sb)
            for _ in range(repeat):
                _emit_body(nc, tc, pools, aps)

    nc.compile()
    return nc


def _get_nc(repeat=1):
    key = ("nc", repeat)
    if key not in _cache:
        _cache[key] = _build(repeat)
    return _cache[key]


def _in_maps(story_a, u, story_c, H):
    story_a = np.ascontiguousarray(story_a, dtype=np.float32)
    u = np.ascontiguousarray(u, dtype=np.float32)
    story_c = np.ascontiguousarray(story_c, dtype=np.float32)
    H = np.ascontiguousarray(H, dtype=np.float32)
    maps = []
    for i in range(N_CORES):
        s = slice(i * BPC, (i + 1) * BPC)
        maps.append({
            "story_a": story_a[s],
            "u": u[s],
            "story_c": story_c[s],
            "H": H,
        })
    return maps


def kernel(story_a, u, story_c, H):
    from concourse.bass_utils import run_bass_kernel_spmd

    nc = _get_nc()
    maps = _in_maps(story_a, u, story_c, H)
    res = run_bass_kernel_spmd(nc, maps, core_ids=list(range(N_CORES)))
    out = np.concatenate([res.results[i]["out"] for i in range(N_CORES)],
                         axis=0)
    return out.astype(np.float32)


# revision 9
# speedup vs baseline: 6.5777x; 1.0199x over previous
"""MemN2N block kernel for 8 TRN2 NeuronCores.

Reference computation (per batch b):
    m      = story_a @ u^T          # (M, Q)  contraction over D
    p      = softmax(m, axis=Q)     # (M, Q)  softmax over Q (free axis)
    c      = p^T @ story_c          # (Q, D)  contraction over M
    out    = c + u @ H              # (Q, D)

Sharding: data-parallel over batch B=32 -> 4 batches per core, no
collectives.  Per core the 4 batches are processed INTERLEAVED: each
"super-tile" step handles one 128-row m-tile from each batch, so the
ACT/DVE softmax ops cover (128, 4, 64) at once (4x fewer instructions,
amortizing the ~200ns fixed cost per op that otherwise dominates).

Per super-tile:
  - 4x PE transpose of story_a tiles into one PSUM bank (D onto parts)
  - one ACT copy PSUM->SBUF of all 4 transposed tiles
  - 4x mm1: scores[:, b, :] = aT_b.T @ uT_b
  - one ACT exp over (128, 4, 64)  (softmax over q needs no
    max-subtraction: |scores| <~ 70 keeps exp inside fp32 range)
  - one DVE reduce_sum -> Z (128, 4), one DVE reciprocal,
    one DVE broadcast-multiply to normalize p
  - 4x mm2: acc[:, b, :] += c_tile_b.T @ p_b   (PSUM accumulation)
Tail per batch: mm3 acc += H.T @ uT (the u @ H residual, same PSUM
accumulation group), PE transpose of acc -> (q, d), DMA out.

M rows are processed in a permuted order (m = p*64 + n) so each DMA
chunk is 8KB-contiguous per partition; softmax is per-row and the
weighted sum over M is order-invariant, so the permutation cancels as
long as story_a and story_c use the same one.
"""

import numpy as np

B, M, Q, D = 32, 8192, 64, 128
N_CORES = 8
BPC = B // N_CORES          # batches per core
P = 128                     # partitions / m-tile rows
NT = M // P                 # 64 m-subtiles per batch
CHUNK = 16                  # m-subtiles per DMA chunk
NCH = NT // CHUNK           # chunks per batch

_cache = {}


def _emit_body(nc, tc, pools, aps, variant="full"):
    """Emit one full pass of the per-core computation.

    variant: "full" | "dma" (loads only) | "nodma" (compute on one
    preloaded chunk) — the non-full variants exist only for benchmark
    attribution experiments.
    """
    from concourse import mybir

    f32 = mybir.dt.float32
    Exp = mybir.ActivationFunctionType.Exp
    consts, io, work, small, ps_t, ps_s, ps_acc = pools
    a_ap, u_ap, c_ap, h_ap, o_ap, ident, h_sb = aps

    # u tiles and their transposes uT_b (D on partitions), loaded once
    uT_sbs = []
    for b in range(BPC):
        u_sb = small.tile([Q, D], f32, tag="u_sb")
        nc.sync.dma_start(u_sb, u_ap[b])
        uT_ps = ps_t.tile([D, Q], f32, tag="at_ps")
        nc.tensor.transpose(uT_ps, u_sb, ident[:Q, :Q])
        uT_sb = small.tile([D, Q], f32, tag=f"uT_sb{b}")
        nc.vector.tensor_copy(uT_sb, uT_ps)
        uT_sbs.append(uT_sb)

    # (M, D) viewed as (p, n, d) with m = p*NT + n
    a_rs = [a_ap[b].rearrange("(p n) d -> p n d", p=P) for b in range(BPC)]
    c_rs = [c_ap[b].rearrange("(p n) d -> p n d", p=P) for b in range(BPC)]

    acc_ps = ps_acc.tile([P, BPC, Q], f32, tag="acc")

    # Software pipeline with stage offsets so the in-order PE queue
    # never waits on the ACT/DVE softmax chain:
    #   iteration i emits  T(i) | mm1(i-1)+softmax(i-1) | mm2(i-2).
    # While PE runs T(i) and mm1(i-1), ACT copies aT(i-1)->SBUF and the
    # ACT/DVE softmax of tile i-2 completes, so mm2(i-2) is ready.
    chunk_tiles = {}      # ch -> (achs, cchs)
    at_sbs = {}           # idx -> transposed story_a tiles in SBUF
    s_pss = {}            # idx -> scores PSUM
    p_sbs = {}            # idx -> normalized softmax in SBUF

    def load_chunk(ch):
        if variant == "nodma" and ch > 0:
            chunk_tiles[ch] = chunk_tiles[0]
            return
        achs, cchs = [], []
        for b in range(BPC):
            ach = io.tile([P, CHUNK, D], f32, tag=f"ach{b}")
            nc.sync.dma_start(ach, a_rs[b][:, ch * CHUNK:(ch + 1) * CHUNK, :])
            achs.append(ach)
            cch = io.tile([P, CHUNK, D], f32, tag=f"cch{b}")
            nc.sync.dma_start(cch, c_rs[b][:, ch * CHUNK:(ch + 1) * CHUNK, :])
            cchs.append(cch)
        chunk_tiles[ch] = (achs, cchs)

    def stage_transpose(idx):
        achs, _ = chunk_tiles[idx // CHUNK]
        j = idx % CHUNK
        at_ps = ps_t.tile([P, BPC, P], f32, tag="at_ps")
        for b in range(BPC):
            nc.tensor.transpose(at_ps[:, b, :], achs[b][:, j, :], ident)
        at_sb = work.tile([P, BPC, P], f32, tag="at_sb")
        nc.scalar.copy(at_sb, at_ps)
        at_sbs[idx] = at_sb

    def stage_scores(idx):
        at_sb = at_sbs.pop(idx)
        s_ps = ps_s.tile([P, BPC, Q], f32, tag="s_ps")
        for b in range(BPC):
            nc.tensor.matmul(s_ps[:, b, :], at_sb[:, b, :], uT_sbs[b],
                             start=True, stop=True)
        p_sb = work.tile([P, BPC, Q], f32, tag="p_sb")
        nc.scalar.activation(p_sb, s_ps, Exp)
        z = small.tile([P, BPC], f32, tag="z")
        nc.vector.tensor_reduce(z, p_sb, mybir.AxisListType.X,
                                mybir.AluOpType.add)
        zi = small.tile([P, BPC], f32, tag="zi")
        nc.vector.reciprocal(zi, z)
        nc.vector.tensor_tensor(p_sb, p_sb,
                                zi[:, :, None].to_broadcast(p_sb.shape),
                                mybir.AluOpType.mult)
        p_sbs[idx] = p_sb

    def stage_weighted_sum(idx):
        _, cchs = chunk_tiles[idx // CHUNK]
        j = idx % CHUNK
        p_sb = p_sbs.pop(idx)
        for b in range(BPC):
            # PSUM accumulation-group state is per bank: only the very
            # first matmul on this bank may use start=True (interleaved
            # per-batch groups reset each other's has_written bits and
            # drop contributions).  The first write of every other batch
            # region lands on cleared has_written bits and overwrites,
            # so start=False is correct there too.
            nc.tensor.matmul(acc_ps[:, b, :], cchs[b][:, j, :],
                             p_sb[:, b, :],
                             start=(idx == 0 and b == 0), stop=False,
                             skip_group_check=True)

    for i in range(NT + 2):
        if i < NT:
            if i % CHUNK == 0:
                load_chunk(i // CHUNK)
            if variant == "dma":
                continue
            stage_transpose(i)
        if variant == "dma":
            continue
        if 1 <= i <= NT:
            stage_scores(i - 1)
        if i >= 2:
            stage_weighted_sum(i - 2)
    if variant == "dma":
        return

    # residual: acc_b += H.T @ uT_b  == (u_b @ H)^T
    for b in range(BPC):
        nc.tensor.matmul(acc_ps[:, b, :], h_sb, uT_sbs[b],
                         start=False, stop=(b == BPC - 1),
                         skip_group_check=True)

    acc_sb = work.tile([P, BPC, Q], f32, tag="acc_sb")
    nc.scalar.copy(acc_sb, acc_ps)
    oT_ps = ps_t.tile([Q, BPC, D], f32, tag="at_ps")
    for b in range(BPC):
        nc.tensor.transpose(oT_ps[:, b, :], acc_sb[:, b, :], ident)
    o_sb = work.tile([Q, BPC, D], f32, tag="o_sb")
    nc.vector.tensor_copy(o_sb, oT_ps)
    for b in range(BPC):
        nc.sync.dma_start(o_ap[b], o_sb[:, b, :])


def _build(repeat=1, variant="full"):
    import concourse.tile as tile
    from concourse import bacc, mybir
    from concourse.masks import make_identity

    f32 = mybir.dt.float32
    nc = bacc.Bacc("TRN2", target_bir_lowering=False, debug=False,
                   num_devices=N_CORES)

    a_ap = nc.dram_tensor("story_a", [BPC, M, D], f32, kind="ExternalInput").ap()
    u_ap = nc.dram_tensor("u", [BPC, Q, D], f32, kind="ExternalInput").ap()
    c_ap = nc.dram_tensor("story_c", [BPC, M, D], f32, kind="ExternalInput").ap()
    h_ap = nc.dram_tensor("H", [D, D], f32, kind="ExternalInput").ap()
    o_ap = nc.dram_tensor("out", [BPC, Q, D], f32, kind="ExternalOutput").ap()

    with tile.TileContext(nc) as tc:
        with (
            tc.tile_pool(name="consts", bufs=1) as consts,
            tc.tile_pool(name="io", bufs=2) as io,
            tc.tile_pool(name="work", bufs=3) as work,
            tc.tile_pool(name="small", bufs=4) as small,
            tc.tile_pool(name="ps_t", bufs=3, space="PSUM") as ps_t,
            tc.tile_pool(name="ps_s", bufs=3, space="PSUM") as ps_s,
            tc.tile_pool(name="ps_acc", bufs=2, space="PSUM") as ps_acc,
        ):
            ident = consts.tile([P, P], f32)
            make_identity(nc, ident)
            h_sb = consts.tile([D, D], f32)
            nc.sync.dma_start(h_sb, h_ap)

            pools = (consts, io, work, small, ps_t, ps_s, ps_acc)
            aps = (a_ap, u_ap, c_ap, h_ap, o_ap, ident, h_sb)
            for _ in range(repeat):
                _emit_body(nc, tc, pools, aps, variant=variant)

    nc.compile()
    return nc


def _get_nc(repeat=1, variant="full"):
    key = ("nc", repeat, variant)
    if key not in _cache:
        _cache[key] = _build(repeat, variant)
    return _cache[key]


def _in_maps(story_a, u, story_c, H):
    story_a = np.ascontiguousarray(story_a, dtype=np.float32)
    u = np.ascontiguousarray(u, dtype=np.float32)
    story_c = np.ascontiguousarray(story_c, dtype=np.float32)
    H = np.ascontiguousarray(H, dtype=np.float32)
    maps = []
    for i in range(N_CORES):
        s = slice(i * BPC, (i + 1) * BPC)
        maps.append({
            "story_a": story_a[s],
            "u": u[s],
            "story_c": story_c[s],
            "H": H,
        })
    return maps


def kernel(story_a, u, story_c, H):
    from concourse.bass_utils import run_bass_kernel_spmd

    nc = _get_nc()
    maps = _in_maps(story_a, u, story_c, H)
    res = run_bass_kernel_spmd(nc, maps, core_ids=list(range(N_CORES)))
    out = np.concatenate([res.results[i]["out"] for i in range(N_CORES)],
                         axis=0)
    return out.astype(np.float32)
